# revision 5
# baseline (speedup 1.0000x reference)
"""MultiHeadCrossAttention TRN2 kernel.

Problem: B=4, S=2048, D=1024, H=16 heads, HD=64.
  kv = x@Wkv+bkv; q = y@Wq+bq; per head: softmax(q k^T/8 + mask) @ v; @Wo+bo.

Sharding (8 cores): core = (batch b, head-group g of 8 heads).  Each core
computes a partial output for its batch from its 8 heads; the host sums the
two head-group partials per batch and adds bo.

Per-core dataflow (all matmuls contract along SBUF partitions):
  - host pre-transposes x,y -> xT,yT [D,S]; scales Wq,bq by 1/sqrt(HD)
  - kT[d,s], qT[d,s] computed head-transposed (2 heads packed per 128
    partitions); v computed seq-major [s, G*(HD+1)] with a per-head "ones"
    column (via a zero weight column + bias 1.0) so the attention matmul's
    65th output row accumulates the softmax denominator for free.
  - kT/qT are stored fp16 (post-projection values are O(1), so fp16's
    ~5e-4 relative rounding is harmless; halves SBUF and matmul runs at
    1 cycle/row).  Scores are computed k-major: S^T[k,q] = kT^T qT per
    (head, kblock).
    No row-max subtraction (logits are bounded ~|8|; exp is fp32-safe);
    mask is folded in as exp(s+m) = exp(s)*exp(m) with exp(maskT) fp16
    precomputed once on device into DRAM scratch.
  - attention A = exp(S^T) * expmask (fp16, DVE 2x) feeds vals^T = v'^T A
    accumulated in PSUM; row 64 = denominators.  vals^T / denom -> f32r.
  - out = vals @ Wo accumulated per 128-row q block, streamed to DRAM.

Matmul dtype: float32r (1 cycle/row for N>=256, ~1.5e-4 rel err) for all
f32 operand matmuls; fp16 for the attention-probability matmul.
"""

import numpy as np

import concourse.bass as bass
import concourse.tile as tile
from concourse import bacc, mybir
from concourse.bass_utils import run_bass_kernel_spmd

F32 = mybir.dt.float32
F32R = mybir.dt.float32r
F16 = mybir.dt.float16
EXP = mybir.ActivationFunctionType.Exp

B, S, D, H, HD = 4, 2048, 1024, 16, 64
NCORES = 8
G = H // 2  # heads per core


def build_program(s=S, d=D, g=G, hd=HD, reps=1):
    """Per-core Bass program. s: seq len, d: model dim, g: heads/core."""
    nP = g // 2          # head pairs (2 heads of hd=64 pack 128 partitions)
    nC = d // 128        # contraction chunks
    nKB = s // 128       # key blocks
    nQB = max(1, s // 512)
    qw = min(512, s)     # q tile width
    gw = g * hd          # head-group width (k/q columns per core)
    vw = g * (hd + 1)    # v' width incl. ones columns
    vh = vw // 2         # v' half width (fits one PSUM bank, >=256 for f32r)
    assert hd == 64 and d % 128 == 0 and s % 128 == 0

    nc = bacc.Bacc("TRN2", target_bir_lowering=False, debug=False)

    xT = nc.dram_tensor("xT", [d, s], F32R, kind="ExternalInput").ap()
    yT = nc.dram_tensor("yT", [d, s], F32R, kind="ExternalInput").ap()
    maskT = nc.dram_tensor("maskT", [s, s], F32, kind="ExternalInput").ap()
    wk = nc.dram_tensor("wk", [d, gw], F32R, kind="ExternalInput").ap()
    wq = nc.dram_tensor("wq", [d, gw], F32R, kind="ExternalInput").ap()
    wvp = nc.dram_tensor("wvp", [d, vw], F32R, kind="ExternalInput").ap()
    wo = nc.dram_tensor("wo", [gw, d], F32R, kind="ExternalInput").ap()
    bk = nc.dram_tensor("bk", [gw], F32, kind="ExternalInput").ap()
    bq = nc.dram_tensor("bq", [gw], F32, kind="ExternalInput").ap()
    bvp = nc.dram_tensor("bvp", [vw], F32, kind="ExternalInput").ap()
    out = nc.dram_tensor("out", [s, d], F32, kind="ExternalOutput").ap()

    # waves of kblocks sharing one PSUM scores tile / ACT exp pass
    waves = []
    kb0 = 0
    while kb0 < nKB:
        waves.append((kb0, min(3, nKB - kb0)))
        kb0 += 3
    ww = min(3, nKB)  # max wave size

    with tile.TileContext(nc) as tc:
        with (
            tc.tile_pool(name="persist", bufs=1) as persist,
            tc.tile_pool(name="dram", bufs=1, space="DRAM") as dram,
        ):
            expmask = dram.tile([s, s], F16)

            def body():
                # ---- weights / biases to SBUF ----
                wk_sb = persist.tile([128, nC * gw], F32R)
                wq_sb = persist.tile([128, nC * gw], F32R)
                wvp_sb = persist.tile([128, nC * vw], F32R)
                wo_sb = persist.tile([128, nP * d], F32R)
                bk_sb = persist.tile([128, nP], F32)
                bq_sb = persist.tile([128, nP], F32)
                bvp1 = persist.tile([1, vw], F32)
                bvp_sb = persist.tile([128, vw], F32)
                for c in range(nC):
                    nc.sync.dma_start(wk_sb[:, c * gw:(c + 1) * gw],
                                      wk[c * 128:(c + 1) * 128, :])
                    nc.sync.dma_start(wq_sb[:, c * gw:(c + 1) * gw],
                                      wq[c * 128:(c + 1) * 128, :])
                    nc.sync.dma_start(wvp_sb[:, c * vw:(c + 1) * vw],
                                      wvp[c * 128:(c + 1) * 128, :])
                for c in range(nP):
                    nc.sync.dma_start(wo_sb[:, c * d:(c + 1) * d],
                                      wo[c * 128:(c + 1) * 128, :])
                nc.sync.dma_start(bk_sb[:], bk.rearrange("(c p) -> p c", p=128))
                nc.sync.dma_start(bq_sb[:], bq.rearrange("(c p) -> p c", p=128))
                nc.sync.dma_start(bvp1[:], bvp[None, :])
                nc.gpsimd.partition_broadcast(bvp_sb[:], bvp1[:], channels=128)

                kt = persist.tile([128, nP * s], F16)
                qt = persist.tile([128, nP * s], F16)
                vp = persist.tile([128, nKB * vw], F16)

                # ---- phase 0: expmask = exp(maskT) -> DRAM fp16 ----
                with (
                    tc.tile_pool(name="ph0", bufs=2) as ph0,
                    tc.tile_pool(name="xy", bufs=1) as xy,
                    tc.tile_pool(name="ph1ps", bufs=4, space="PSUM") as ph1ps,
                ):
                    for i in range(nKB):
                        mt = ph0.tile([128, s], F32, bufs=2)
                        nc.sync.dma_start(mt[:], maskT[i * 128:(i + 1) * 128, :])
                        et = ph0.tile([128, s], F16, bufs=2)
                        nc.scalar.activation(et[:], mt[:], EXP)
                        nc.sync.dma_start(expmask[i * 128:(i + 1) * 128, :], et[:])

                    # ---- phase 1: projections ----
                    for src, wsb, bsb, dst in ((xT, wk_sb, bk_sb, kt),
                                               (yT, wq_sb, bq_sb, qt)):
                        xyt = xy.tile([128, nC * s], F32R, tag="xyt", bufs=1,
                                      name="xyt")
                        for c in range(nC):
                            nc.sync.dma_start(xyt[:, c * s:(c + 1) * s],
                                              src[c * 128:(c + 1) * 128, :])
                        for p in range(nP):
                            for q0 in range(0, s, 512):
                                ps = ph1ps.tile([128, 512], F32, tag="pskq",
                                                bufs=4, name="ps")
                                w = min(512, s - q0)
                                for c in range(nC):
                                    nc.tensor.matmul(
                                        ps[:, :w],
                                        wsb[:, c * gw + p * 128:
                                            c * gw + (p + 1) * 128],
                                        xyt[:, c * s + q0:c * s + q0 + w],
                                        start=(c == 0), stop=(c == nC - 1))
                                nc.vector.tensor_scalar_add(
                                    dst[:, p * s + q0:p * s + q0 + w],
                                    ps[:, :w], bsb[:, p:p + 1])
                        if src is xT:  # v' projection off xT
                            for sb in range(nKB):
                                for hf in range(2):
                                    ps2 = ph1ps.tile([128, vh], F32, tag="psv",
                                                     bufs=4, name="ps2")
                                    for c in range(nC):
                                        nc.tensor.matmul(
                                            ps2[:],
                                            xyt[:, c * s + sb * 128:
                                                c * s + (sb + 1) * 128],
                                            wvp_sb[:, c * vw + hf * vh:
                                                   c * vw + (hf + 1) * vh],
                                            start=(c == 0), stop=(c == nC - 1))
                                    nc.vector.tensor_tensor(
                                        vp[:, sb * vw + hf * vh:
                                           sb * vw + (hf + 1) * vh],
                                        ps2[:], bvp_sb[:, hf * vh:(hf + 1) * vh],
                                        op=mybir.AluOpType.add)

                # ---- phase 2: attention ----
                with (
                    tc.tile_pool(name="ph2", bufs=2) as ph2,
                    tc.tile_pool(name="sps", bufs=2, space="PSUM") as spsum,
                    tc.tile_pool(name="vps", bufs=2, space="PSUM") as vpsum,
                ):
                    for qb in range(nQB):
                        q0 = qb * qw
                        ms = ph2.tile([128, nKB * qw], F16, tag="ms", bufs=2,
                                      name="ms")
                        for kb in range(nKB):
                            nc.sync.dma_start(
                                ms[:, kb * qw:(kb + 1) * qw],
                                expmask[kb * 128:(kb + 1) * 128, q0:q0 + qw])
                        vtq = ph2.tile([128, nP * qw], F32R, tag="vtq", bufs=2,
                                       name="vtq")
                        for h in range(g):
                            p, r0 = h // 2, (h % 2) * 64
                            vps = vpsum.tile([hd + 1, qw], F32, tag="vps",
                                             bufs=2, name="vps")
                            for w0, wn in waves:
                                sp = spsum.tile([128, ww * qw], F32, tag="sp",
                                                bufs=2, name="sp")
                                for i in range(wn):
                                    kb = w0 + i
                                    nc.tensor.matmul(
                                        sp[:, i * qw:(i + 1) * qw],
                                        kt[r0:r0 + hd,
                                           p * s + kb * 128:p * s + (kb + 1) * 128],
                                        qt[r0:r0 + hd, p * s + q0:p * s + q0 + qw],
                                        start=True, stop=True)
                                et = ph2.tile([128, ww * qw], F16, tag="et",
                                              bufs=2, name="et")
                                nc.scalar.activation(et[:, :wn * qw],
                                                     sp[:, :wn * qw], EXP)
                                at = ph2.tile([128, ww * qw], F16, tag="at",
                                              bufs=2, name="at")
                                nc.vector.tensor_tensor(
                                    at[:, :wn * qw], et[:, :wn * qw],
                                    ms[:, w0 * qw:(w0 + wn) * qw],
                                    op=mybir.AluOpType.mult)
                                for i in range(wn):
                                    kb = w0 + i
                                    nc.tensor.matmul(
                                        vps[:],
                                        vp[:, kb * vw + h * (hd + 1):
                                           kb * vw + (h + 1) * (hd + 1)],
                                        at[:, i * qw:(i + 1) * qw],
                                        start=(kb == 0), stop=(kb == nKB - 1))
                            rc = ph2.tile([1, qw], F32, tag="rc", bufs=2,
                                          name="rc")
                            nc.vector.reciprocal(rc[:], vps[hd:hd + 1, :])
                            rb = ph2.tile([hd, qw], F32, tag="rb", bufs=2,
                                          name="rb")
                            nc.gpsimd.partition_broadcast(rb[:], rc[:],
                                                          channels=hd)
                            nc.vector.tensor_tensor(
                                vtq[r0:r0 + hd, p * qw:(p + 1) * qw],
                                vps[:hd, :], rb[:], op=mybir.AluOpType.mult)

                        # ---- phase 3: output projection for this q block ----
                        jw = min(512, d)
                        for m in range(qw // 128):
                            op = spsum.tile([128, ww * qw], F32, tag="sp",
                                            bufs=2, name="op")
                            for j in range(d // jw):
                                for c in range(nP):
                                    nc.tensor.matmul(
                                        op[:, j * jw:(j + 1) * jw],
                                        vtq[:, c * qw + m * 128:
                                            c * qw + (m + 1) * 128],
                                        wo_sb[:, c * d + j * jw:
                                              c * d + (j + 1) * jw],
                                        start=(c == 0), stop=(c == nP - 1))
                            ot = ph2.tile([128, d], F32, tag="ot", bufs=2,
                                          name="ot")
                            nc.vector.tensor_copy(ot[:], op[:, :d])
                            nc.sync.dma_start(
                                out[q0 + m * 128:q0 + (m + 1) * 128, :], ot[:])

            if reps == 1:
                body()
            else:
                with tc.For_i(0, reps, 1):
                    body()

    nc.compile()
    return nc


def host_prep(x, y, mask, Wkv, bkv, Wq, bq, Wo, bo, s=S, d=D, g=G, hd=HD):
    """Build the 8 per-core input maps."""
    gw, vw = g * hd, g * (hd + 1)
    scale = 1.0 / np.sqrt(np.float32(hd))
    maskT = np.ascontiguousarray(mask[0, 0].T).astype(np.float32)
    in_maps = []
    xTs = [np.ascontiguousarray(x[b].T) for b in range(x.shape[0])]
    yTs = [np.ascontiguousarray(y[b].T) for b in range(x.shape[0])]
    for core in range(2 * x.shape[0]):
        b, grp = core // 2, core % 2
        c0 = grp * gw
        # Wkv columns are per-head interleaved: head hh owns cols
        # [hh*2*hd, hh*2*hd+hd) = k and [hh*2*hd+hd, (hh+1)*2*hd) = v.
        wk = np.zeros((d, gw), np.float32)
        bk = np.zeros((gw,), np.float32)
        wvp = np.zeros((d, vw), np.float32)
        bvp = np.zeros((vw,), np.float32)
        for h in range(g):
            hh = grp * g + h
            wk[:, h * hd:(h + 1) * hd] = Wkv[:, hh * 2 * hd:hh * 2 * hd + hd]
            bk[h * hd:(h + 1) * hd] = bkv[hh * 2 * hd:hh * 2 * hd + hd]
            wvp[:, h * (hd + 1):h * (hd + 1) + hd] = \
                Wkv[:, hh * 2 * hd + hd:(hh + 1) * 2 * hd]
            bvp[h * (hd + 1):h * (hd + 1) + hd] = \
                bkv[hh * 2 * hd + hd:(hh + 1) * 2 * hd]
            bvp[h * (hd + 1) + hd] = 1.0
        in_maps.append({
            "xT": xTs[b],
            "yT": yTs[b],
            "maskT": maskT,
            "wk": wk,
            "wq": np.ascontiguousarray(Wq[:, c0:c0 + gw]) * scale,
            "wvp": wvp,
            "wo": np.ascontiguousarray(Wo[c0:c0 + gw, :]),
            "bk": bk,
            "bq": np.ascontiguousarray(bq[c0:c0 + gw]) * scale,
            "bvp": bvp,
        })
    return in_maps


_cache = {}


def kernel(x, y, mask, Wkv, bkv, Wq, bq, Wo, bo):
    x = np.asarray(x, np.float32)
    y = np.asarray(y, np.float32)
    mask = np.asarray(mask, np.float32)
    if "nc" not in _cache:
        _cache["nc"] = build_program()
    nc = _cache["nc"]
    in_maps = host_prep(x, y, mask,
                        np.asarray(Wkv, np.float32), np.asarray(bkv, np.float32),
                        np.asarray(Wq, np.float32), np.asarray(bq, np.float32),
                        np.asarray(Wo, np.float32), np.asarray(bo, np.float32))
    res = run_bass_kernel_spmd(nc, in_maps, list(range(NCORES))).results
    bo = np.asarray(bo, np.float32)
    return np.stack([res[2 * b]["out"] + res[2 * b + 1]["out"] + bo
                     for b in range(B)])


# revision 7
# speedup vs baseline: 1.1899x; 1.1899x over previous
"""MultiHeadCrossAttention TRN2 kernel.

Problem: B=4, S=2048, D=1024, H=16 heads, HD=64.
  kv = x@Wkv+bkv; q = y@Wq+bq; per head: softmax(q k^T/8 + mask) @ v; @Wo+bo.

Sharding (8 cores): core = (batch b, head-group g of 8 heads).  Each core
computes a partial output for its batch from its 8 heads; the host sums the
two head-group partials per batch and adds bo.

Per-core dataflow (all matmuls contract along SBUF partitions):
  - host pre-transposes x,y -> xT,yT [D,S]; scales Wq,bq by 1/sqrt(HD)
  - kT[d,s], qT[d,s] computed head-transposed (2 heads packed per 128
    partitions); v computed seq-major [s, G*(HD+1)] with a per-head "ones"
    column (via a zero weight column + bias 1.0) so the attention matmul's
    65th output row accumulates the softmax denominator for free.
  - kT/qT are stored fp16 (post-projection values are O(1), so fp16's
    ~5e-4 relative rounding is harmless; halves SBUF and matmul runs at
    1 cycle/row).  Scores are computed k-major: S^T[k,q] = kT^T qT per
    (head, kblock).
    No row-max subtraction (logits are bounded ~|8|; exp is fp32-safe);
    mask is folded in as exp(s+m) = exp(s)*exp(m) with exp(maskT) fp16
    precomputed once on device into DRAM scratch.
  - attention A = exp(S^T) * expmask (fp16, DVE 2x) feeds vals^T = v'^T A
    accumulated in PSUM; row 64 = denominators.  vals^T / denom -> f32r.
  - out = vals @ Wo accumulated per 128-row q block, streamed to DRAM.

Matmul dtype: float32r (1 cycle/row for N>=256, ~1.5e-4 rel err) for all
f32 operand matmuls; fp16 for the attention-probability matmul.
"""

import numpy as np

import concourse.bass as bass
import concourse.tile as tile
from concourse import bacc, mybir
from concourse.bass_utils import run_bass_kernel_spmd

F32 = mybir.dt.float32
F32R = mybir.dt.float32r
F16 = mybir.dt.float16
EXP = mybir.ActivationFunctionType.Exp

B, S, D, H, HD = 4, 2048, 1024, 16, 64
NCORES = 8
G = H // 2  # heads per core


def build_program(s=S, d=D, g=G, hd=HD, reps=1):
    """Per-core Bass program. s: seq len, d: model dim, g: heads/core."""
    nP = g // 2          # head pairs (2 heads of hd=64 pack 128 partitions)
    nC = d // 128        # contraction chunks
    nKB = s // 128       # key blocks
    nQB = max(1, s // 512)
    qw = min(512, s)     # q tile width
    gw = g * hd          # head-group width (k/q columns per core)
    vw = g * (hd + 1)    # v' width incl. ones columns
    vh = vw // 2         # v' half width (fits one PSUM bank, >=256 for f32r)
    assert hd == 64 and d % 128 == 0 and s % 128 == 0

    nc = bacc.Bacc("TRN2", target_bir_lowering=False, debug=False)

    xT = nc.dram_tensor("xT", [d, s], F32R, kind="ExternalInput").ap()
    yT = nc.dram_tensor("yT", [d, s], F32R, kind="ExternalInput").ap()
    maskT = nc.dram_tensor("maskT", [s, s], F32, kind="ExternalInput").ap()
    wk = nc.dram_tensor("wk", [d, gw], F32R, kind="ExternalInput").ap()
    wq = nc.dram_tensor("wq", [d, gw], F32R, kind="ExternalInput").ap()
    wvp = nc.dram_tensor("wvp", [d, vw], F32R, kind="ExternalInput").ap()
    wo = nc.dram_tensor("wo", [gw, d], F16, kind="ExternalInput").ap()
    bk = nc.dram_tensor("bk", [gw], F32, kind="ExternalInput").ap()
    bq = nc.dram_tensor("bq", [gw], F32, kind="ExternalInput").ap()
    bvp = nc.dram_tensor("bvp", [vw], F32, kind="ExternalInput").ap()
    out = nc.dram_tensor("out", [s, d], F32, kind="ExternalOutput").ap()

    # waves of kblocks sharing one PSUM scores tile / ACT exp pass
    waves = []
    kb0 = 0
    while kb0 < nKB:
        waves.append((kb0, min(3, nKB - kb0)))
        kb0 += 3
    ww = min(3, nKB)  # max wave size

    with tile.TileContext(nc) as tc:
        with tc.tile_pool(name="persist", bufs=1) as persist:
            def body():
                # ---- weights / biases to SBUF ----
                wk_sb = persist.tile([128, nC * gw], F32R)
                wq_sb = persist.tile([128, nC * gw], F32R)
                wvp_sb = persist.tile([128, nC * vw], F32R)
                wo_sb = persist.tile([128, nP * d], F16)
                bk_sb = persist.tile([128, nP], F32)
                bq_sb = persist.tile([128, nP], F32)
                bvp1 = persist.tile([1, vw], F32)
                bvp_sb = persist.tile([128, vw], F32)
                nc.sync.dma_start(wk_sb[:], wk.rearrange("(c p) n -> p c n", p=128))
                nc.sync.dma_start(wq_sb[:], wq.rearrange("(c p) n -> p c n", p=128))
                nc.sync.dma_start(wvp_sb[:], wvp.rearrange("(c p) n -> p c n", p=128))
                nc.sync.dma_start(wo_sb[:], wo.rearrange("(c p) n -> p c n", p=128))
                nc.sync.dma_start(bk_sb[:], bk.rearrange("(c p) -> p c", p=128))
                nc.sync.dma_start(bq_sb[:], bq.rearrange("(c p) -> p c", p=128))
                nc.sync.dma_start(bvp1[:], bvp[None, :])
                nc.gpsimd.partition_broadcast(bvp_sb[:], bvp1[:], channels=128)

                kt = persist.tile([128, nP * s], F16)
                qt = persist.tile([128, nP * s], F16)
                vp = persist.tile([128, nKB * vw], F16)

                # ---- phase 1: projections ----
                with (
                    tc.tile_pool(name="xy", bufs=1) as xy,
                    tc.tile_pool(name="ph1ps", bufs=4, space="PSUM") as ph1ps,
                ):
                    for src, wsb, bsb, dst in ((xT, wk_sb, bk_sb, kt),
                                               (yT, wq_sb, bq_sb, qt)):
                        xyt = xy.tile([128, nC * s], F32R, tag="xyt", bufs=1,
                                      name="xyt")
                        nc.sync.dma_start(
                            xyt[:], src.rearrange("(c p) n -> p c n", p=128))
                        for p in range(nP):
                            for q0 in range(0, s, 512):
                                ps = ph1ps.tile([128, 512], F32, tag="pskq",
                                                bufs=4, name="ps")
                                w = min(512, s - q0)
                                for c in range(nC):
                                    nc.tensor.matmul(
                                        ps[:, :w],
                                        wsb[:, c * gw + p * 128:
                                            c * gw + (p + 1) * 128],
                                        xyt[:, c * s + q0:c * s + q0 + w],
                                        start=(c == 0), stop=(c == nC - 1))
                                nc.vector.tensor_scalar_add(
                                    dst[:, p * s + q0:p * s + q0 + w],
                                    ps[:, :w], bsb[:, p:p + 1])
                        if src is xT:  # v' projection off xT
                            for sb in range(nKB):
                                for hf in range(2):
                                    ps2 = ph1ps.tile([128, vh], F32, tag="psv",
                                                     bufs=4, name="ps2")
                                    for c in range(nC):
                                        nc.tensor.matmul(
                                            ps2[:],
                                            xyt[:, c * s + sb * 128:
                                                c * s + (sb + 1) * 128],
                                            wvp_sb[:, c * vw + hf * vh:
                                                   c * vw + (hf + 1) * vh],
                                            start=(c == 0), stop=(c == nC - 1))
                                    nc.vector.tensor_tensor(
                                        vp[:, sb * vw + hf * vh:
                                           sb * vw + (hf + 1) * vh],
                                        ps2[:], bvp_sb[:, hf * vh:(hf + 1) * vh],
                                        op=mybir.AluOpType.add)

                # ---- phase 2: attention ----
                with (
                    tc.tile_pool(name="ph2", bufs=2) as ph2,
                    tc.tile_pool(name="sps", bufs=2, space="PSUM") as spsum,
                    tc.tile_pool(name="vps", bufs=2, space="PSUM") as vpsum,
                ):
                    for qb in range(nQB):
                        q0 = qb * qw
                        ms = ph2.tile([128, nKB * qw], F16, tag="ms", bufs=2,
                                      name="ms")
                        mq = 4 if nKB % 4 == 0 else 1  # kblocks per mask DMA
                        for j in range(nKB // mq):
                            mtl = ph2.tile([128, mq * qw], F32, tag="mtl",
                                           bufs=2, name="mtl")
                            nc.sync.dma_start(
                                mtl[:],
                                maskT[j * mq * 128:(j + 1) * mq * 128,
                                      q0:q0 + qw].rearrange(
                                          "(b p) q -> p b q", p=128))
                            nc.scalar.activation(
                                ms[:, j * mq * qw:(j + 1) * mq * qw], mtl[:],
                                EXP)
                        vtq = ph2.tile([128, nP * qw], F16, tag="vtq", bufs=2,
                                       name="vtq")
                        for h in range(g):
                            p, r0 = h // 2, (h % 2) * 64
                            vps = vpsum.tile([hd + 1, qw], F32, tag="vps",
                                             bufs=2, name="vps")
                            for w0, wn in waves:
                                sp = spsum.tile([128, ww * qw], F32, tag="sp",
                                                bufs=2, name="sp")
                                for i in range(wn):
                                    kb = w0 + i
                                    nc.tensor.matmul(
                                        sp[:, i * qw:(i + 1) * qw],
                                        kt[r0:r0 + hd,
                                           p * s + kb * 128:p * s + (kb + 1) * 128],
                                        qt[r0:r0 + hd, p * s + q0:p * s + q0 + qw],
                                        start=True, stop=True)
                                et = ph2.tile([128, ww * qw], F16, tag="et",
                                              bufs=2, name="et")
                                nc.scalar.activation(et[:, :wn * qw],
                                                     sp[:, :wn * qw], EXP)
                                at = ph2.tile([128, ww * qw], F16, tag="at",
                                              bufs=2, name="at")
                                nc.vector.tensor_tensor(
                                    at[:, :wn * qw], et[:, :wn * qw],
                                    ms[:, w0 * qw:(w0 + wn) * qw],
                                    op=mybir.AluOpType.mult)
                                for i in range(wn):
                                    kb = w0 + i
                                    nc.tensor.matmul(
                                        vps[:],
                                        vp[:, kb * vw + h * (hd + 1):
                                           kb * vw + (h + 1) * (hd + 1)],
                                        at[:, i * qw:(i + 1) * qw],
                                        start=(kb == 0), stop=(kb == nKB - 1))
                            rc = ph2.tile([1, qw], F32, tag="rc", bufs=2,
                                          name="rc")
                            nc.vector.reciprocal(rc[:], vps[hd:hd + 1, :])
                            rb = ph2.tile([hd, qw], F32, tag="rb", bufs=2,
                                          name="rb")
                            nc.gpsimd.partition_broadcast(rb[:], rc[:],
                                                          channels=hd)
                            nc.vector.tensor_tensor(
                                vtq[r0:r0 + hd, p * qw:(p + 1) * qw],
                                vps[:hd, :], rb[:], op=mybir.AluOpType.mult)

                        # ---- phase 3: output projection for this q block ----
                        jw = min(512, d)
                        for m in range(qw // 128):
                            op = spsum.tile([128, ww * qw], F32, tag="sp",
                                            bufs=2, name="op")
                            for j in range(d // jw):
                                for c in range(nP):
                                    nc.tensor.matmul(
                                        op[:, j * jw:(j + 1) * jw],
                                        vtq[:, c * qw + m * 128:
                                            c * qw + (m + 1) * 128],
                                        wo_sb[:, c * d + j * jw:
                                              c * d + (j + 1) * jw],
                                        start=(c == 0), stop=(c == nP - 1))
                            ot = ph2.tile([128, d], F32, tag="ot", bufs=2,
                                          name="ot")
                            nc.vector.tensor_copy(ot[:], op[:, :d])
                            nc.sync.dma_start(
                                out[q0 + m * 128:q0 + (m + 1) * 128, :], ot[:])

            if reps == 1:
                body()
            else:
                with tc.For_i(0, reps, 1):
                    body()

    nc.compile()
    return nc


def host_prep(x, y, mask, Wkv, bkv, Wq, bq, Wo, bo, s=S, d=D, g=G, hd=HD):
    """Build the 8 per-core input maps."""
    gw, vw = g * hd, g * (hd + 1)
    scale = 1.0 / np.sqrt(np.float32(hd))
    maskT = np.ascontiguousarray(mask[0, 0].T).astype(np.float32)
    in_maps = []
    xTs = [np.ascontiguousarray(x[b].T) for b in range(x.shape[0])]
    yTs = [np.ascontiguousarray(y[b].T) for b in range(x.shape[0])]
    for core in range(2 * x.shape[0]):
        b, grp = core // 2, core % 2
        c0 = grp * gw
        # Wkv columns are per-head interleaved: head hh owns cols
        # [hh*2*hd, hh*2*hd+hd) = k and [hh*2*hd+hd, (hh+1)*2*hd) = v.
        wk = np.zeros((d, gw), np.float32)
        bk = np.zeros((gw,), np.float32)
        wvp = np.zeros((d, vw), np.float32)
        bvp = np.zeros((vw,), np.float32)
        for h in range(g):
            hh = grp * g + h
            wk[:, h * hd:(h + 1) * hd] = Wkv[:, hh * 2 * hd:hh * 2 * hd + hd]
            bk[h * hd:(h + 1) * hd] = bkv[hh * 2 * hd:hh * 2 * hd + hd]
            wvp[:, h * (hd + 1):h * (hd + 1) + hd] = \
                Wkv[:, hh * 2 * hd + hd:(hh + 1) * 2 * hd]
            bvp[h * (hd + 1):h * (hd + 1) + hd] = \
                bkv[hh * 2 * hd + hd:(hh + 1) * 2 * hd]
            bvp[h * (hd + 1) + hd] = 1.0
        in_maps.append({
            "xT": xTs[b],
            "yT": yTs[b],
            "maskT": maskT,
            "wk": wk,
            "wq": np.ascontiguousarray(Wq[:, c0:c0 + gw]) * scale,
            "wvp": wvp,
            "wo": np.ascontiguousarray(Wo[c0:c0 + gw, :]).astype(np.float16),
            "bk": bk,
            "bq": np.ascontiguousarray(bq[c0:c0 + gw]) * scale,
            "bvp": bvp,
        })
    return in_maps


_cache = {}


def kernel(x, y, mask, Wkv, bkv, Wq, bq, Wo, bo):
    x = np.asarray(x, np.float32)
    y = np.asarray(y, np.float32)
    mask = np.asarray(mask, np.float32)
    if "nc" not in _cache:
        _cache["nc"] = build_program()
    nc = _cache["nc"]
    in_maps = host_prep(x, y, mask,
                        np.asarray(Wkv, np.float32), np.asarray(bkv, np.float32),
                        np.asarray(Wq, np.float32), np.asarray(bq, np.float32),
                        np.asarray(Wo, np.float32), np.asarray(bo, np.float32))
    res = run_bass_kernel_spmd(nc, in_maps, list(range(NCORES))).results
    bo = np.asarray(bo, np.float32)
    return np.stack([res[2 * b]["out"] + res[2 * b + 1]["out"] + bo
                     for b in range(B)])
